# revision 20
# baseline (speedup 1.0000x reference)
"""Trainium2 Bass kernel for nn_Kalman_filter_34041910788634.

Mathematical collapse of the reference:
  - The scan's step() ignores its carry (st, e_t = inp rebinds both from the
    scan inputs), and the parameter-network output o is time-invariant, so the
    whole T_LEN-step loop reduces to evaluating the last step (T[-1], e[-1]).
  - The second MLP matmul (h @ W2.T, 34 GFLOP) is only consumed through dot
    products with e8 and T8, so it collapses to h @ (W2.T @ e8) and
    h[0] @ (W2.T @ T8): two matvecs.

Device work: the one big matmul h.T = relu(W1 @ Q.T + b1) [4096, 2048], run
in fp8(e4m3) DoubleRow mode - 256-deep contraction per pass, which measures
at the full 2x PE rate (216 ns per [256x128x512] matmul, fp8 peak).  e4m3
operand quantization gives ~8.8e-3 rel err on the final output, inside the
2e-2 gate (validated numerically against the fixed-seed inputs).  h comes
back as bf16; the hidden-dim contraction with ve (8 MFLOP) and the matvec
glue run on the host around the sharded launch.

Sharding: 2-way over Q rows x 4-way over the hidden dim (core k handles row
half k//4, hidden quarter k%4).  Versus pure hidden-sharding this halves the
replicated Q.T stream per core (2 MB fp8), which matters because each DMA
queue only sustains ~60-190 GB/s (line-size dependent): pass 1's demand then
fits in the two HWDGE rings, streamed kt-major in exact consumption order
with W1 interleaved.  The GpSimd SWDGE queue opens ~4 us late and is only
fast with >=8 KB lines, so it carries bulk with no early deadline: pass-2
weights and the pass-1 h writeback.  Q.T stays SBUF-resident: pass 2
(hidden blocks 4-7) re-reads SBUF group-at-a-time so relus and writebacks
overlap remaining matmuls.  Junk matmuls on the first weight chunk pre-warm
the PE HAM clock gate; a dummy activation preloads the Relu LUT early.
"""

import os
import sys

for _p in ("/opt/trn_rl_repo", "/root/.axon_site/_ro/trn_rl_repo"):
    if os.path.isdir(_p) and _p not in sys.path:
        sys.path.insert(0, _p)

import ml_dtypes
import numpy as np

import concourse.bass as bass
import concourse.bass2jax as _bass2jax
import concourse.mybir as mybir
import concourse.tile as tile
from concourse.bass_utils import run_bass_kernel_spmd


def _split_multiwaits(bir_bytes):
    """The walrus build in this container supports at most one sync-wait
    condition per instruction; Tile freely emits several.  Hoist extra waits
    onto NoOp instructions inserted just before the owning instruction (same
    engine, so per-engine program order makes this equivalent)."""
    import orjson

    b = orjson.loads(bir_bytes)
    n = 0
    for func in b.get("functions", []):
        for blk in func.get("blocks", []):
            newl = []
            for ins in blk.get("instructions", []):
                si = ins.get("sync_info")
                ws = (si or {}).get("on_wait") or []
                if len(ws) > 1:
                    for wv in ws[:-1]:
                        n += 1
                        newl.append({
                            "debug": ins.get("debug", 0),
                            "engine": ins["engine"],
                            "ins": [],
                            "outs": [],
                            "name": f"{ins['name']}-wsplit{n}",
                            "opcode": "NoOp",
                            "sync_info": {"on_update": [], "on_wait": [wv]},
                        })
                    si["on_wait"] = ws[-1:]
                newl.append(ins)
            blk["instructions"] = newl
    return orjson.dumps(b)


_orig_compile_bir_kernel = _bass2jax.compile_bir_kernel


def _patched_compile_bir_kernel(ant_bir_str, compile_dir, neff_name="file.neff"):
    return _orig_compile_bir_kernel(
        _split_multiwaits(ant_bir_str), compile_dir, neff_name=neff_name
    )


if _bass2jax.compile_bir_kernel is not _patched_compile_bir_kernel:
    _bass2jax.compile_bir_kernel = _patched_compile_bir_kernel


N_DIM = 2048
HIDDEN = 4096
OUT_DIM = 512
NCORES = 8
HQ = 4                      # hidden quarters
RH = 2                      # row halves
JQ = HIDDEN // HQ           # 1024 hidden units per core
JB = JQ // 128              # 8 stationary column blocks per core
KT = N_DIM // 256           # 8 k-pair chunks (256-deep DoubleRow contraction)
RC = 2                      # 2 moving-dim chunks of 512 (per row half)

SQ = 32.0                   # fp8 scale for Q
SW = 128.0                  # fp8 scale for W1
SH = 1.0                    # h writeback is bf16: no extra scaling
HSCALE = SH / (SQ * SW)

F8 = mybir.dt.float8e4
BF = mybir.dt.bfloat16
F32 = mybir.dt.float32
RELU = mybir.ActivationFunctionType.Relu
DR = mybir.MatmulPerfMode.DoubleRow

PREWARM = 6
PREWARM32 = 24

# Per-HWDGE-ring streams, in consumption order.  DMA queues move one
# per-partition contiguous line per ~15-20 ns, so bandwidth scales with
# line length: each ("b", a, lo, hi) item merges the pass-1 weight pair a
# (j-blocks 2a, 2a+1) WITH its Q.T column block a for k-chunks [lo,hi)
# into single 1536 B/partition/chunk lines, laid out so matmul operands
# are direct slices: [.., 0:128] / [.., 128:256] = the two lhsT blocks,
# [.., 256:768] = the moving Q.T block.  The GP queue (slow start, needs
# big lines) carries pass-2 weights as two 4 KB-line slabs ("W2", b:
# j-block pair 4+2b over all kt).
SCHED = {
    "sp": [("b", 0, 0, 1), ("b", 0, 1, 2), ("b", 0, 2, 4),
           ("b", 0, 4, 6), ("b", 0, 6, 8)],
    "act": [("b", 1, 0, 1), ("b", 1, 1, 2), ("b", 1, 2, 4),
            ("b", 1, 4, 6), ("b", 1, 6, 8)],
    "gp": [("W2", 0, 0, 8), ("W2", 1, 0, 8)],
}


def _item_len(it):
    kind, a, lo, hi = it
    return (hi - lo) * (1536 if kind == "b" else 512)


_cache = {}


def _build_nc():
    nc = bass.Bass(target_bir_lowering=False)

    slabs = {q: nc.dram_tensor(f"slab_{q}", [128, sum(_item_len(i) for i in items)],
                               F8, kind="ExternalInput")
             for q, items in SCHED.items()}
    b1c = nc.dram_tensor("b1c", [128, JB], F32, kind="ExternalInput")
    # hout[p, jb, rc, n]; 16 KB/partition total so the pass-1 half can leave
    # on the GP queue as one 8 KB-line DMA.
    hout = nc.dram_tensor("hout", [128, JB, RC, 512], BF, kind="ExternalOutput")

    with tile.TileContext(nc) as tc:
        with (
            tc.tile_pool(name="qpool", bufs=1) as qpool,
            tc.tile_pool(name="small", bufs=1) as small,
            tc.tile_pool(name="h4pool", bufs=1) as h4pool,
            tc.tile_pool(name="hpool", bufs=2) as hpool,
            tc.tile_pool(name="psh", bufs=1, space="PSUM") as psh,
        ):
            b1c_s = small.tile([128, JB], F32, name="b1c_s")
            nc.scalar.dma_start(b1c_s[:], b1c[:])

            engs = {"sp": nc.sync, "act": nc.scalar, "gp": nc.gpsimd}
            qmap, wmap = {}, {}
            for q, items in SCHED.items():
                off = 0
                for it in items:
                    kind, a, lo, hi = it
                    L = _item_len(it)
                    if kind == "b":
                        t = qpool.tile([128, hi - lo, 2, 768], F8,
                                       name=f"b_{a}_{lo}")
                        for kt in range(lo, hi):
                            qmap[(kt, a)] = (t, kt - lo)
                            wmap[(kt, 2 * a)] = ("b", t, kt - lo, 0)
                            wmap[(kt, 2 * a + 1)] = ("b", t, kt - lo, 1)
                    else:
                        jb0 = 4 + 2 * a
                        t = qpool.tile([128, hi - lo, 2, 2, 128], F8,
                                       name=f"{kind}_{a}_{lo}")
                        for kt in range(lo, hi):
                            wmap[(kt, jb0)] = ("W2", t, kt - lo, 0)
                            wmap[(kt, jb0 + 1)] = ("W2", t, kt - lo, 1)
                    engs[q].dma_start(t[:], slabs[q][:, off:off + L])
                    off += L

            def qt(kt, rc):
                t, i = qmap[(kt, rc)]
                return t[:, i, :, 256:768]

            def w1(kt, jb):
                kind, t, i, j = wmap[(kt, jb)]
                if kind == "b":
                    return t[:, i, :, 128 * j:128 * (j + 1)]
                return t[:, i, j]

            # Preload the Relu LUT while DMAs stream so the first real
            # activation doesn't pay the ~1.3us ACT_TABLE_LOAD.
            dum = small.tile([128, 1], F32, name="dum")
            nc.scalar.activation(dum[:], b1c_s[:, 0:1], RELU)

            phs = {}
            for i in range(4):
                for rc in range(RC):
                    phs[(i, rc)] = psh.tile([128, 512], F32,
                                            name=f"ps1_{i}_{rc}",
                                            tag=f"ps_{i}_{rc}")

            # Prewarm: junk matmuls start the PE's HAM busy-window during
            # the DMA fill.  b1c (2 KB, first on the ACT ring) arrives ~2us
            # before the first fp8 chunk, so warm on it in fp32 first.
            for _ in range(PREWARM32):
                nc.tensor.matmul(phs[(0, 0)][0:8, 0:8], b1c_s[:, 0:8],
                                 b1c_s[:, 0:8], start=True, stop=True)
            for _ in range(PREWARM):
                nc.tensor.matmul(phs[(0, 0)][:, 0:128], w1(0, 0), w1(0, 0),
                                 start=True, stop=True, perf_mode=DR)

            # Pass 1 (j-blocks 0-3): k-chunk outer so consumption tracks
            # the two HWDGE streams.  After each of the early blocks, pad
            # the PE with junk LDWEIGHTS (no psum side effects) so DMA
            # micro-stalls don't reset the HAM busy window and re-throttle
            # the clock; the next real matmul reloads its own weights.
            for kt in range(KT):
                for i in range(4):
                    for rc in range(RC):
                        nc.tensor.matmul(
                            phs[(i, rc)][:],
                            w1(kt, i),
                            qt(kt, rc),
                            start=(kt == 0),
                            stop=(kt == KT - 1),
                            perf_mode=DR,
                        )
                if kt < 4:
                    for _ in range(5):
                        nc.tensor.ldweights(w1(0, 0), perf_mode=DR)
            # relu all 8 pass-1 psums into one quad tile; write back as two
            # 4 KB-line halves, one per HWDGE ring.
            h4 = h4pool.tile([128, 4, RC, 512], BF, name="h4")
            for i in range(4):
                for rc in range(RC):
                    nc.scalar.activation(h4[:, i, rc], phs[(i, rc)][:], RELU,
                                         bias=b1c_s[:, i:i + 1],
                                         scale=HSCALE)
            nc.sync.dma_start(hout[:, 0:2], h4[:, 0:2])
            nc.scalar.dma_start(hout[:, 2:4], h4[:, 2:4])

            # Pass 2 (j-blocks 4-7): group-at-a-time (k-chunk inner) so each
            # psum group closes early and its relu+writeback overlap the
            # remaining matmuls; only the last block's drain is exposed.
            for i in range(4):
                jb = 4 + i
                p2 = {}
                # For the last block, run rc1's group first so its relu and
                # writeback overlap rc0's matmuls; only rc0's drain is then
                # exposed after the final matmul.
                rcs = (1, 0) if jb == 7 else (0, 1)
                hr = hpool.tile([128, RC, 512], BF, name="hr", tag="hr")
                for rc in rcs:
                    p2[rc] = psh.tile([128, 512], F32, name=f"ps2_{i}_{rc}",
                                      tag=f"ps_{i}_{rc}")
                    for kt in range(KT):
                        nc.tensor.matmul(
                            p2[rc][:],
                            w1(kt, jb),
                            qt(kt, rc),
                            start=(kt == 0),
                            stop=(kt == KT - 1),
                            perf_mode=DR,
                        )
                    if jb == 7:
                        nc.scalar.activation(hr[:, rc], p2[rc][:], RELU,
                                             bias=b1c_s[:, jb:jb + 1],
                                             scale=HSCALE)
                        if rc == 1:
                            nc.sync.dma_start(hout[:, jb, rc], hr[:, rc])
                        else:
                            nc.scalar.dma_start(hout[:, jb, rc, 0:256],
                                                hr[:, rc, 0:256])
                            nc.sync.dma_start(hout[:, jb, rc, 256:512],
                                              hr[:, rc, 256:512])
                if jb < 7:
                    for rc in rcs:
                        nc.scalar.activation(hr[:, rc], p2[rc][:], RELU,
                                             bias=b1c_s[:, jb:jb + 1],
                                             scale=HSCALE)
                    eng = engs["sp"] if jb % 2 == 0 else engs["act"]
                    eng.dma_start(hout[:, jb], hr[:])

    return nc


def _get_nc():
    if "nc" not in _cache:
        _cache["nc"] = _build_nc()
    return _cache["nc"]


def _to_e4m3(x, scale):
    y = np.clip(np.asarray(x, np.float32) * scale, -240.0, 240.0)
    return y.astype(ml_dtypes.float8_e4m3)


def _pack_slabs(Q5, W5):
    """Per-queue [128, L] byte streams from Q5 [kt, rc, p, t, n] (this
    core's row half) and W5 [kt, jb, p, t, m] (this core's hidden
    quarter)."""
    out = {}
    for q, items in SCHED.items():
        parts = []
        for kind, a, lo, hi in items:
            if kind == "b":
                we = W5[lo:hi, 2 * a].transpose(1, 0, 2, 3)     # [p,nkt,2,128]
                wo = W5[lo:hi, 2 * a + 1].transpose(1, 0, 2, 3)
                qq = Q5[lo:hi, a].transpose(1, 0, 2, 3)         # [p,nkt,2,512]
                blk = np.concatenate([we, wo, qq], axis=3)      # [p,nkt,2,768]
            else:
                jb0 = 4 + 2 * a
                blk = W5[lo:hi, jb0:jb0 + 2].transpose(2, 0, 1, 3, 4)
            parts.append(np.ascontiguousarray(blk).reshape(128, -1))
        out[q] = np.ascontiguousarray(np.concatenate(parts, axis=1))
    return out


def kernel(**inputs):
    T = np.asarray(inputs["T"], np.float32)
    e = np.asarray(inputs["e"], np.float32)
    w = np.asarray(inputs["w"], np.float32)
    Q = np.asarray(inputs["Q"], np.float32)
    W1 = np.asarray(inputs["W1"], np.float32)
    b1 = np.asarray(inputs["b1"], np.float32)
    W2 = np.asarray(inputs["W2"], np.float32)
    b2 = np.asarray(inputs["b2"], np.float32)
    fc_w = np.asarray(inputs["fc_w"], np.float32)
    fc_b = np.asarray(inputs["fc_b"], np.float32)

    T8 = T[-1]
    e8 = e[-1]

    # Q5h[rh][kt, rc, p, t, n] = Qs[rh*1024 + rc*512 + n, kt*256 + t*128 + p]
    Qs = _to_e4m3(Q, SQ)
    Q5h = [np.ascontiguousarray(
        Qs[rh * 1024:(rh + 1) * 1024]
        .reshape(RC, 512, KT, 2, 128).transpose(2, 0, 4, 3, 1))
        for rh in range(RH)]
    # W5q[hq][kt, jb, p, t, m] = W1s[hq*1024 + jb*128 + m, kt*256 + t*128 + p]
    W5q = [np.ascontiguousarray(
        _to_e4m3(W1[hq * JQ:(hq + 1) * JQ, :], SW)
        .reshape(JB, 128, KT, 2, 128).transpose(2, 0, 4, 3, 1))
        for hq in range(HQ)]
    ve = e8 @ W2                                        # [4096] = W2.T @ e8
    vT = T8 @ W2

    in_maps = []
    for k in range(NCORES):
        rh, hq = k // HQ, k % HQ
        m = {f"slab_{q}": s for q, s in _pack_slabs(Q5h[rh], W5q[hq]).items()}
        m["b1c"] = np.ascontiguousarray(
            b1[hq * JQ:(hq + 1) * JQ].reshape(JB, 128).T) * np.float32(SH)
        in_maps.append(m)

    res = run_bass_kernel_spmd(_get_nc(), in_maps, core_ids=list(range(NCORES))).results

    # aq[rh*1024 + rc*512 + n] += sum_{jb,p} hout[p,jb,rc,n]*ve[hq*1024+jb*128+p]
    aQ = np.zeros(N_DIM, np.float64)
    for k in range(NCORES):
        rh, hq = k // HQ, k % HQ
        hk = np.asarray(res[k]["hout"]).astype(np.float32)
        vek = ve[hq * JQ:(hq + 1) * JQ].reshape(JB, 128).astype(np.float64) / SH
        aQ[rh * 1024:(rh + 1) * 1024] += np.einsum(
            "pjrn,jp->rn", hk.astype(np.float64), vek).reshape(-1)

    # Host-side glue (tiny BLAS-1/2): Qe, hw row, scalars, final fc.
    Qe = (Q.astype(np.float64) @ e8.astype(np.float64))
    hw = np.maximum(W1.astype(np.float64) @ w.astype(np.float64)
                    + b1.astype(np.float64), 0.0)
    g0 = float(hw @ vT.astype(np.float64))
    p_wst = float(w.astype(np.float64) @ T8.astype(np.float64)) + g0 \
        + float(b2.astype(np.float64) @ T8.astype(np.float64))
    st = p_wst + Qe + aQ + float(b2.astype(np.float64) @ e8.astype(np.float64))
    out = st.astype(np.float32) @ fc_w.T + fc_b
    return out.astype(np.float32)
